# revision 6
# baseline (speedup 1.0000x reference)
"""HalfKP input layer (embedding_lookup) on 8 Trainium2 NeuronCores.

Reference computation (B=1024, K=64, F=640, C=256):
    p = piece_positions.reshape(B, 640).astype(f32)          # values in {0,1}
    Wg = input_weights[king_positions]                       # (B, 2, 641, 256)
    out[b] = sum_f p[b,f] * (Wg[b,0,f,:] + Wg[b,1,f,:])
             + Wg[b,0,640,:] + Wg[b,1,640,:] + bias

Strategy — SINGLE launch, channel-sharded (launch fixed cost on this stack
is ~13-16us per launch, so the two-launch host-routed design pays that
twice):
  * Core c owns output channels [32c, 32c+32).  It computes ALL 2048
    (sample, king-slot) pair rows restricted to its 32 channels, so the
    pair combine (rowA + rowB) is core-local — no cross-core traffic.
  * Weight table read stays minimal: each core reads only its 32-channel
    slice of the table (bf16), 2.62 MB/core -> the full table is read
    exactly once in aggregate.  bf16-only weights keep max-rel-err ~2e-3,
    well under the 2e-2 gate.
  * The 2048 pairs are grouped by king into column-contiguous runs packed
    exactly into 4 PSUM bins of 512 columns (groups split at 256-column
    DMA-piece boundaries).  Compute is transposed: psum[32ch, col] +=
    W[k,ch].T @ feats, one matmul per (subgroup, feature-chunk); a final
    K=2 matmul per subgroup adds the row-640 extra (all pairs) and bias
    (slot-A pairs only).
  * All DMAs ride the two HWDGE queues with >=2.5KB contiguous
    per-partition runs (a uint8->bf16 SWDGE cast variant was 2.5x slower:
    2560-descriptor ring storm + a ~57us ring drain before the gather).
  * Pair combine: one GPSIMD ap_gather (~0.4us measured) pulls rowA|rowB
    columns into pair order, one DVE add produces outT[32, 1024].  The
    host transposes the per-core slices back into (1024, 256) — pure
    indexing.
"""

import os
from contextlib import ExitStack

import numpy as np
import ml_dtypes

import concourse.bass as bass
import concourse.tile as tile
from concourse import bacc, mybir
from concourse.bass_utils import run_bass_kernel_spmd

B = 1024
K = 64
F = 640
C = 256
NCORES = 8
CH = C // NCORES       # 32 channels per core
FCH = F // 128         # 5 feature chunks of 128
P = 128
NCOL = 2 * B           # 2048 pair columns
BINW = 512             # one PSUM bank of fp32
NBIN = NCOL // BINW    # 4
PIECEW = 256           # feats DMA piece width (columns)
NPIECE = NCOL // PIECEW  # 8
WCHUNK = 8             # kings per weight DMA chunk
NWCH = K // WCHUNK     # 8

BF16 = ml_dtypes.bfloat16

# Exposed for test harnesses
LAST_RESULTS = []
LAST_EXEC_NS = None

_cache = {}


def _prep(king_positions):
    """Group the 2048 (sample, slot) pairs by king into a 2048-long column
    order, split groups exactly at 256-column DMA-piece boundaries."""
    kings = np.asarray(king_positions).astype(np.int64)  # (B, 2)
    groups = [[] for _ in range(K)]
    for b in range(B):
        groups[kings[b, 0]].append((b, 0))
        groups[kings[b, 1]].append((b, 1))

    order = []       # col -> (b, s)
    subgroups = []   # (king, c0, n)
    col = 0
    for k in range(K):
        g = groups[k]
        i = 0
        while i < len(g):
            room = PIECEW - (col % PIECEW)
            n = min(len(g) - i, room)
            subgroups.append((k, col, n))
            order.extend(g[i : i + n])
            i += n
            col += n
    assert col == NCOL

    pos = np.empty((B, 2), dtype=np.int64)
    for c, (b, s) in enumerate(order):
        pos[b, s] = c
    return order, tuple(subgroups), pos


def _build(subgroups):
    nc = bacc.Bacc(
        "TRN2", target_bir_lowering=False, debug=False, num_devices=NCORES
    )
    dt = mybir.dt

    w_in = nc.dram_tensor("w_in", [P, K, FCH, CH], dt.bfloat16, kind="ExternalInput")
    feats = nc.dram_tensor(
        "feats", [P, NPIECE, FCH, PIECEW], dt.bfloat16, kind="ExternalInput"
    )
    eb = nc.dram_tensor("eb", [2, K * CH], dt.bfloat16, kind="ExternalInput")
    flags = nc.dram_tensor("flags", [2, NCOL], dt.bfloat16, kind="ExternalInput")
    idx = nc.dram_tensor("idx", [CH, NCOL // 16], dt.int16, kind="ExternalInput")
    outT = nc.dram_tensor("outT", [CH, B], dt.float32, kind="ExternalOutput")

    with tile.TileContext(nc) as tc, ExitStack() as ctx:
        const_pool = ctx.enter_context(tc.tile_pool(name="const", bufs=1))
        psum_pool = ctx.enter_context(tc.tile_pool(name="psum", bufs=1, space="PSUM"))

        # small tensors first (cheap, unblock the K=2 extras matmuls)
        idx_sb = const_pool.tile([CH, NCOL // 16], dt.int16)
        nc.sync.dma_start(out=idx_sb[:], in_=idx.ap())
        flags_sb = const_pool.tile([2, NCOL], dt.bfloat16)
        nc.scalar.dma_start(out=flags_sb[:], in_=flags.ap())
        eb_sb = const_pool.tile([2, K * CH], dt.bfloat16)
        nc.sync.dma_start(out=eb_sb[:], in_=eb.ap())

        # weights (8-king chunks) and features (256-col pieces), interleaved
        # across the two HWDGE queues so chunk i and piece i land together
        w_sb = []
        f_sb = []
        for i in range(NPIECE):
            wt = const_pool.tile([P, WCHUNK * FCH * CH], dt.bfloat16, tag="w")
            (nc.sync if i % 2 == 0 else nc.scalar).dma_start(
                out=wt[:],
                in_=w_in[:, i * WCHUNK : (i + 1) * WCHUNK, :, :].rearrange(
                    "p k f c -> p (k f c)"
                ),
            )
            w_sb.append(wt)
            ft = const_pool.tile([P, FCH * PIECEW], dt.bfloat16, tag="f")
            (nc.scalar if i % 2 == 0 else nc.sync).dma_start(
                out=ft[:].rearrange("p (f c) -> p f c", f=FCH),
                in_=feats[:, i, :, :],
            )
            f_sb.append(ft)

        psum_t = [
            psum_pool.tile([P, BINW], dt.float32, space="PSUM", name=f"acc{i}")
            for i in range(NBIN)
        ]

        for k, c0, n in subgroups:
            bin_ = c0 // BINW
            off = c0 % BINW
            piece = c0 // PIECEW
            poff = c0 % PIECEW
            wc, kk = divmod(k, WCHUNK)
            for ch in range(FCH):
                nc.tensor.matmul(
                    out=psum_t[bin_][0:CH, off : off + n],
                    lhsT=w_sb[wc][:, (kk * FCH + ch) * CH : (kk * FCH + ch + 1) * CH],
                    rhs=f_sb[piece][:, ch * PIECEW + poff : ch * PIECEW + poff + n],
                    start=(ch == 0),
                    stop=False,
                )
            # row-640 extra for every pair + bias on slot-A pairs (K=2)
            nc.tensor.matmul(
                out=psum_t[bin_][0:CH, off : off + n],
                lhsT=eb_sb[0:2, k * CH : (k + 1) * CH],
                rhs=flags_sb[0:2, c0 : c0 + n],
                start=False,
                stop=True,
            )

        rows_sb = const_pool.tile([CH, NCOL], dt.float32)
        for bin_ in range(NBIN):
            nc.vector.tensor_copy(
                rows_sb[:, bin_ * BINW : (bin_ + 1) * BINW], psum_t[bin_][0:CH, :]
            )

        gat_sb = const_pool.tile([CH, NCOL], dt.float32)
        nc.gpsimd.ap_gather(
            out_ap=gat_sb[:],
            in_ap=rows_sb[:],
            idxs_ap=idx_sb[:],
            channels=CH,
            num_elems=NCOL,
            d=1,
            num_idxs=NCOL,
        )
        outT_sb = const_pool.tile([CH, B], dt.float32)
        nc.vector.tensor_add(outT_sb[:], gat_sb[:, 0:B], gat_sb[:, B : 2 * B])
        nc.sync.dma_start(out=outT.ap(), in_=outT_sb[:])

    nc.compile()
    return nc


def kernel(piece_positions, king_positions, input_weights, bias):
    global LAST_RESULTS, LAST_EXEC_NS

    p_flat = np.asarray(piece_positions).reshape(B, F)
    w_full = np.ascontiguousarray(np.asarray(input_weights), dtype=np.float32)
    bias_np = np.asarray(bias, dtype=np.float32)

    order, subgroups, pos = _prep(king_positions)

    if subgroups not in _cache:
        _cache[subgroups] = _build(subgroups)
    nc = _cache[subgroups]

    w_bf = w_full.astype(BF16)  # (K, 641, C)

    # features in pair-column order: (2048, 640) -> (128, 8, 5, 256) bf16
    bs = np.array([b for b, _ in order], dtype=np.int64)
    feats = p_flat[bs].astype(np.float32).reshape(NCOL, FCH, 128).transpose(2, 1, 0)
    feats = (
        feats.reshape(128, FCH, NPIECE, PIECEW)
        .transpose(0, 2, 1, 3)
        .astype(BF16)
    )
    feats = np.ascontiguousarray(feats)

    # flags: row0 = 1 (row-640 extra), row1 = slot-A indicator (bias once)
    flags = np.zeros((2, NCOL), dtype=np.float32)
    flags[0, :] = 1.0
    flags[1, :] = np.array([1.0 if s == 0 else 0.0 for _, s in order])
    flags = flags.astype(BF16)

    # gather indices: [idxA | idxB], wrapped in 16 partitions, replicated
    idxall = np.concatenate([pos[:, 0], pos[:, 1]]).astype(np.int16)  # (2048,)
    idx_w = np.zeros((CH, NCOL // 16), dtype=np.int16)
    wrap = idxall.reshape(NCOL // 16, 16).T  # [p, col]
    for blk in range(CH // 16):
        idx_w[blk * 16 : (blk + 1) * 16, :] = wrap

    in_maps = []
    for c in range(NCORES):
        chs = slice(c * CH, (c + 1) * CH)
        w_c = (
            w_bf[:, :F, chs].reshape(K, FCH, 128, CH).transpose(2, 0, 1, 3)
        )  # (128, K, FCH, CH)
        eb_c = np.zeros((2, K, CH), dtype=np.float32)
        eb_c[0] = w_full[:, F, chs]
        eb_c[1] = bias_np[chs][None, :]
        in_maps.append(
            {
                "w_in": np.ascontiguousarray(w_c),
                "feats": feats,
                "eb": np.ascontiguousarray(eb_c.reshape(2, K * CH)).astype(BF16),
                "flags": flags,
                "idx": idx_w,
            }
        )

    do_trace = bool(int(os.environ.get("KERNEL_TRACE", "0")))
    trace_kw = dict(
        trace=do_trace, trace_cores=list(range(NCORES)) if do_trace else None
    )

    res = run_bass_kernel_spmd(nc, in_maps, list(range(NCORES)), **trace_kw)

    LAST_RESULTS = [res]
    LAST_EXEC_NS = res.exec_time_ns

    out = np.empty((B, C), dtype=np.float32)
    for c in range(NCORES):
        out[:, c * CH : (c + 1) * CH] = res.results[c]["outT"].T
    return out


# revision 8
# speedup vs baseline: 1.1788x; 1.1788x over previous
"""HalfKP input layer (embedding_lookup) on 8 Trainium2 NeuronCores.

Reference computation (B=1024, K=64, F=640, C=256):
    p = piece_positions.reshape(B, 640).astype(f32)          # values in {0,1}
    Wg = input_weights[king_positions]                       # (B, 2, 641, 256)
    out[b] = sum_f p[b,f] * (Wg[b,0,f,:] + Wg[b,1,f,:])
             + Wg[b,0,640,:] + Wg[b,1,640,:] + bias

Strategy — SINGLE launch, channel-sharded (launch fixed cost on this stack
is ~13-16us, so the two-launch host-routed design pays it twice):
  * Core c owns output channels [32c, 32c+32).  It computes ALL 2048
    (sample, king-slot) pair rows restricted to its 32 channels, so the
    pair combine (rowA + rowB) is core-local — no cross-core traffic.
  * Table read stays minimal: each core reads only its 32-channel bf16
    slice (2.62 MB) -> the table is read once in aggregate.
  * HWDGE descriptor generation paces DMAs at ~40ns/descriptor
    (one descriptor per partition-run), so all bulk tensors use
    >=6.8KB contiguous per-partition runs: weights and features each in
    3 pieces on the two HWDGE queues -> HBM-bound (~358 GB/s), pieces
    land staggered so matmuls pipeline behind the stream.
  * Compute is transposed: psum[32ch, col] += W[k,ch].T @ feats.  The
    2048 king-grouped pair columns live in ONE psum tile [128, 512]:
    band b = global cols [512b, 512b+512) at partitions [32b, 32b+32).
    Col-tiled (tile_position=(0,32b)) matmuls from different bands run
    concurrently in the PE array.
  * The row-640 extra + bias are NOT matmuls: a GPSIMD ap_gather pulls
    w640[king(col)] (+bias on slot-A) from a small on-device-built table
    into an extras plane, folded in by the per-band PSUM->SBUF add.
  * Pair combine: one ap_gather (measured ~0.4us) into [rowA | rowB]
    order + one DVE add -> outT[32, 1024].  Host transposes per-core
    slices into (1024, 256) — pure indexing.
"""

import os
from contextlib import ExitStack

import numpy as np
import ml_dtypes

import concourse.bass as bass
import concourse.tile as tile
from concourse import bacc, mybir
from concourse.bass_utils import run_bass_kernel_spmd

B = 1024
K = 64
F = 640
C = 256
NCORES = 8
CH = C // NCORES       # 32 channels per core
FCH = F // 128         # 5 feature chunks of 128
P = 128
NCOL = 2 * B           # 2048 pair columns
BINW = 512             # psum band width (one fp32 bank)
NBAND = NCOL // BINW   # 4
NPIECE = 3             # DMA pieces for weights/features
# feature-piece column edges and weight-piece king edges
FEDGE = [0, 683, 1366, 2048]
KEDGE = [0, 22, 43, 64]

BF16 = ml_dtypes.bfloat16

# Exposed for test harnesses
LAST_RESULTS = []
LAST_EXEC_NS = None

_cache = {}


def _prep(king_positions):
    """Group the 2048 (sample, slot) pairs by king into a 2048-long column
    order; split groups at psum-band (512) and DMA-piece edges."""
    kings = np.asarray(king_positions).astype(np.int64)  # (B, 2)
    groups = [[] for _ in range(K)]
    for b in range(B):
        groups[kings[b, 0]].append((b, 0))
        groups[kings[b, 1]].append((b, 1))

    edges = sorted(set([b * BINW for b in range(NBAND + 1)] + FEDGE))

    order = []       # col -> (b, s)
    subgroups = []   # (king, c0, n)
    col = 0
    for k in range(K):
        g = groups[k]
        i = 0
        while i < len(g):
            nxt = min(e for e in edges if e > col)
            n = min(len(g) - i, nxt - col)
            subgroups.append((k, col, n))
            order.extend(g[i : i + n])
            i += n
            col += n
    assert col == NCOL

    pos = np.empty((B, 2), dtype=np.int64)
    for c, (b, s) in enumerate(order):
        pos[b, s] = c
    return order, tuple(subgroups), pos


def _wrap16(idx_flat, nch):
    """ap_gather index layout: wrapped in 16 partitions, replicated per
    16-channel block."""
    n = idx_flat.shape[0]
    out = np.zeros((nch, n // 16), dtype=np.int16)
    wrap = idx_flat.astype(np.int16).reshape(n // 16, 16).T
    for blk in range(nch // 16):
        out[blk * 16 : (blk + 1) * 16, :] = wrap
    return out


def _build(subgroups):
    nc = bacc.Bacc(
        "TRN2", target_bir_lowering=False, debug=False, num_devices=NCORES
    )
    dt = mybir.dt

    w_ins = [
        nc.dram_tensor(
            f"w_in{i}", [P, KEDGE[i + 1] - KEDGE[i], FCH, CH], dt.bfloat16,
            kind="ExternalInput",
        )
        for i in range(NPIECE)
    ]
    f_ins = [
        nc.dram_tensor(
            f"f_in{i}", [P, FCH, FEDGE[i + 1] - FEDGE[i]], dt.bfloat16,
            kind="ExternalInput",
        )
        for i in range(NPIECE)
    ]
    w640T_d = nc.dram_tensor("w640T", [CH, K], dt.float32, kind="ExternalInput")
    bias_d = nc.dram_tensor("bias_t", [CH, 1], dt.float32, kind="ExternalInput")
    idx_d = nc.dram_tensor("idx", [CH, NCOL // 16], dt.int16, kind="ExternalInput")
    idx2_d = nc.dram_tensor("idx2", [CH, NCOL // 16], dt.int16, kind="ExternalInput")
    outT = nc.dram_tensor("outT", [CH, B], dt.float32, kind="ExternalOutput")

    with tile.TileContext(nc) as tc, ExitStack() as ctx:
        const_pool = ctx.enter_context(tc.tile_pool(name="const", bufs=1))
        psum_pool = ctx.enter_context(tc.tile_pool(name="psum", bufs=1, space="PSUM"))

        # tiny tensors first on both queues
        idx_sb = const_pool.tile([CH, NCOL // 16], dt.int16)
        nc.sync.dma_start(out=idx_sb[:], in_=idx_d.ap())
        idx2_sb = const_pool.tile([CH, NCOL // 16], dt.int16)
        nc.scalar.dma_start(out=idx2_sb[:], in_=idx2_d.ap())
        w640T_sb = const_pool.tile([CH, K], dt.float32)
        nc.sync.dma_start(out=w640T_sb[:], in_=w640T_d.ap())
        bias_sb = const_pool.tile([CH, 1], dt.float32)
        nc.scalar.dma_start(out=bias_sb[:], in_=bias_d.ap())

        # bulk streams: weights on sync, features on scalar (parallel HWDGE
        # generators); piece i of each lands together
        w_sb = []
        f_sb = []
        for i in range(NPIECE):
            nk = KEDGE[i + 1] - KEDGE[i]
            wt = const_pool.tile([P, nk * FCH * CH], dt.bfloat16, name=f"wt{i}")
            nc.sync.dma_start(
                out=wt[:], in_=w_ins[i].ap().rearrange("p k f c -> p (k f c)")
            )
            w_sb.append(wt)
            nf = FEDGE[i + 1] - FEDGE[i]
            ft = const_pool.tile([P, FCH * nf], dt.bfloat16, name=f"ft{i}")
            nc.scalar.dma_start(
                out=ft[:], in_=f_ins[i].ap().rearrange("p f c -> p (f c)")
            )
            f_sb.append(ft)

        # extras table: table[:, 2k] = w640[k], table[:, 2k+1] = w640[k]+bias
        table_sb = const_pool.tile([CH, 2 * K], dt.float32)
        tview = table_sb[:].rearrange("c (k two) -> c k two", two=2)
        nc.vector.tensor_copy(tview[:, :, 0], w640T_sb[:])
        nc.vector.tensor_tensor(
            out=tview[:, :, 1],
            in0=w640T_sb[:],
            in1=bias_sb[:].to_broadcast([CH, K]),
            op=mybir.AluOpType.add,
        )
        extras_sb = const_pool.tile([CH, NCOL], dt.float32)
        nc.gpsimd.ap_gather(
            out_ap=extras_sb[:],
            in_ap=table_sb[:],
            idxs_ap=idx2_sb[:],
            channels=CH,
            num_elems=2 * K,
            d=1,
            num_idxs=NCOL,
        )

        # one psum tile: band b = cols [512b, 512b+512) at partitions 32b+
        acc = psum_pool.tile([P, BINW], dt.float32, space="PSUM")

        # issue order: by piece, interleaving the (<=2) bands of the piece
        def piece_of(c0):
            for i in range(NPIECE):
                if FEDGE[i] <= c0 < FEDGE[i + 1]:
                    return i
            raise AssertionError

        by_piece_band = {}
        for k, c0, n in subgroups:
            by_piece_band.setdefault(
                (piece_of(c0), c0 // BINW), []
            ).append((k, c0, n))

        for pc in range(NPIECE):
            bands = sorted(b for (p, b) in by_piece_band if p == pc)
            lists = [list(by_piece_band[(pc, b)]) for b in bands]
            li = 0
            while any(lists):
                if lists[li % len(lists)]:
                    k, c0, n = lists[li % len(lists)].pop(0)
                    band = c0 // BINW
                    off = c0 % BINW
                    poff = c0 - FEDGE[pc]
                    npc = FEDGE[pc + 1] - FEDGE[pc]
                    wc = next(
                        i for i in range(NPIECE) if KEDGE[i] <= k < KEDGE[i + 1]
                    )
                    kk = k - KEDGE[wc]
                    for ch in range(FCH):
                        nc.tensor.matmul(
                            out=acc[32 * band : 32 * band + CH, off : off + n],
                            lhsT=w_sb[wc][
                                :, (kk * FCH + ch) * CH : (kk * FCH + ch + 1) * CH
                            ],
                            rhs=f_sb[pc][:, ch * npc + poff : ch * npc + poff + n],
                            start=(ch == 0),
                            stop=(ch == FCH - 1),
                            tile_position=(0, 32 * band),
                        )
                li += 1

        # fold psum band + extras into flat rows (partitions 0-31)
        rows_sb = const_pool.tile([CH, NCOL], dt.float32)
        for band in range(NBAND):
            nc.vector.tensor_tensor(
                out=rows_sb[:, band * BINW : (band + 1) * BINW],
                in0=acc[32 * band : 32 * band + CH, :],
                in1=extras_sb[:, band * BINW : (band + 1) * BINW],
                op=mybir.AluOpType.add,
            )

        gat_sb = const_pool.tile([CH, NCOL], dt.float32)
        nc.gpsimd.ap_gather(
            out_ap=gat_sb[:],
            in_ap=rows_sb[:],
            idxs_ap=idx_sb[:],
            channels=CH,
            num_elems=NCOL,
            d=1,
            num_idxs=NCOL,
        )
        outT_sb = const_pool.tile([CH, B], dt.float32)
        nc.vector.tensor_add(outT_sb[:], gat_sb[:, 0:B], gat_sb[:, B : 2 * B])
        nc.sync.dma_start(out=outT.ap(), in_=outT_sb[:])

    nc.compile()
    return nc


def kernel(piece_positions, king_positions, input_weights, bias):
    global LAST_RESULTS, LAST_EXEC_NS

    p_flat = np.asarray(piece_positions).reshape(B, F)
    w_full = np.ascontiguousarray(np.asarray(input_weights), dtype=np.float32)
    bias_np = np.asarray(bias, dtype=np.float32)

    order, subgroups, pos = _prep(king_positions)

    if subgroups not in _cache:
        _cache[subgroups] = _build(subgroups)
    nc = _cache[subgroups]

    w_bf = w_full.astype(BF16)  # (K, 641, C)

    # features in pair-column order: (2048, 640) -> (128, 5, 2048) bf16
    bs = np.array([b for b, _ in order], dtype=np.int64)
    featsT = (
        p_flat[bs].astype(np.float32).reshape(NCOL, FCH, 128).transpose(2, 1, 0)
    ).astype(BF16)
    f_pieces = [
        np.ascontiguousarray(featsT[:, :, FEDGE[i] : FEDGE[i + 1]])
        for i in range(NPIECE)
    ]

    # gather indices
    idx_w = _wrap16(np.concatenate([pos[:, 0], pos[:, 1]]), CH)
    kingcol = np.empty(NCOL, dtype=np.int64)
    isA = np.empty(NCOL, dtype=np.int64)
    for c, (b, s) in enumerate(order):
        kingcol[c] = np.asarray(king_positions)[b, s]
        isA[c] = 1 if s == 0 else 0
    idx2_w = _wrap16(2 * kingcol + isA, CH)

    in_maps = []
    for c in range(NCORES):
        chs = slice(c * CH, (c + 1) * CH)
        w_c = (
            w_bf[:, :F, chs].reshape(K, FCH, 128, CH).transpose(2, 0, 1, 3)
        )  # (128, K, FCH, CH)
        m = {
            "w640T": np.ascontiguousarray(w_full[:, F, chs].T),
            "bias_t": np.ascontiguousarray(bias_np[chs][:, None]),
            "idx": idx_w,
            "idx2": idx2_w,
        }
        for i in range(NPIECE):
            m[f"w_in{i}"] = np.ascontiguousarray(w_c[:, KEDGE[i] : KEDGE[i + 1]])
            m[f"f_in{i}"] = f_pieces[i]
        in_maps.append(m)

    do_trace = bool(int(os.environ.get("KERNEL_TRACE", "0")))
    trace_kw = dict(
        trace=do_trace, trace_cores=list(range(NCORES)) if do_trace else None
    )

    res = run_bass_kernel_spmd(nc, in_maps, list(range(NCORES)), **trace_kw)

    LAST_RESULTS = [res]
    LAST_EXEC_NS = res.exec_time_ns

    out = np.empty((B, C), dtype=np.float32)
    for c in range(NCORES):
        out[:, c * CH : (c + 1) * CH] = res.results[c]["outT"].T
    return out


# revision 12
# speedup vs baseline: 2.5421x; 2.1565x over previous
"""HalfKP input layer (embedding_lookup) on 8 Trainium2 NeuronCores.

Reference computation (B=1024, K=64, F=640, C=256):
    p = piece_positions.reshape(B, 640).astype(f32)          # values in {0,1}
    Wg = input_weights[king_positions]                       # (B, 2, 641, 256)
    out[b] = sum_f p[b,f] * (Wg[b,0,f,:] + Wg[b,1,f,:])
             + Wg[b,0,640,:] + Wg[b,1,640,:] + bias

Strategy — SINGLE launch, channel-sharded (launch fixed cost on this stack
is ~13-16us, so the two-launch host-routed design pays it twice):
  * Core c owns output channels [32c, 32c+32).  It computes ALL 2048
    (sample, king-slot) pair rows restricted to its 32 channels, so the
    pair combine (rowA + rowB) is core-local — no cross-core traffic.
  * Table read stays minimal: each core reads only its 32-channel bf16
    slice (2.62 MB) -> the table is read once in aggregate.
  * HWDGE descriptor generation paces DMAs at ~40ns/descriptor (one per
    partition-run), so weights/features stream in 3 pieces of ~7KB runs
    on the two HWDGE queues (~350 GB/s aggregate); small tensors ride
    the independent SWDGE (gpsimd) queue.
  * Main compute is transposed: psum[32ch, col] += W[k,ch].T @ feats.
    One psum tile [128, 512]: band b = cols [512b, 512b+512) at
    partitions [32b, 32b+32); col-tiled matmuls from different bands run
    concurrently.  A K=2 matmul per subgroup adds the row-640 extra
    (all pairs) + bias (slot-A pairs) from a tiny [2, 64*32] table.
  * Pair combine ON THE PE (GPSIMD ap_gather measured ~27ns/index =
    55us for 2048 — unusable): one-hot selection tiles
    S_c[128pair, 1024sample] = (sampleof == iota) are built by the DVE
    early (hidden under the DMA window); per 128-pair chunk the bf16 row
    block is PE-transposed, then 32 pairing matmuls
    psumOut += rows_c.T @ S_c accumulate rowA+rowB per sample directly,
    4 chunks concurrently via col-tiling; a DVE chain sums the 4 bands.
  * Host transposes per-core outT[32, 1024] slices into (1024, 256) —
    pure indexing.
"""

import os
from contextlib import ExitStack

import numpy as np
import ml_dtypes

import concourse.bass as bass
import concourse.tile as tile
from concourse import bacc, mybir
from concourse.bass_utils import run_bass_kernel_spmd
from concourse.masks import make_identity

B = 1024
K = 64
F = 640
C = 256
NCORES = 8
CH = C // NCORES       # 32 channels per core
FCH = F // 128         # 5 feature chunks of 128
P = 128
NCOL = 2 * B           # 2048 pair columns
BINW = 512             # psum band width (one fp32 bank)
NBAND = NCOL // BINW   # 4
NCHUNK = NCOL // P     # 16 pair chunks for the pairing matmuls
NPIECE = 3             # DMA pieces for weights/features
FEDGE = [0, 683, 1366, 2048]
KEDGE = [0, 22, 43, 64]

BF16 = ml_dtypes.bfloat16

# Exposed for test harnesses
LAST_RESULTS = []
LAST_EXEC_NS = None

_cache = {}


def _prep(king_positions):
    """Group the 2048 (sample, slot) pairs by king into a 2048-long column
    order; split groups at psum-band (512) and DMA-piece edges."""
    kings = np.asarray(king_positions).astype(np.int64)  # (B, 2)
    groups = [[] for _ in range(K)]
    for b in range(B):
        groups[kings[b, 0]].append((b, 0))
        groups[kings[b, 1]].append((b, 1))

    edges = sorted(set([b * BINW for b in range(NBAND + 1)] + FEDGE))

    order = []       # col -> (b, s)
    subgroups = []   # (king, c0, n)
    col = 0
    for k in range(K):
        g = groups[k]
        i = 0
        while i < len(g):
            nxt = min(e for e in edges if e > col)
            n = min(len(g) - i, nxt - col)
            subgroups.append((k, col, n))
            order.extend(g[i : i + n])
            i += n
            col += n
    assert col == NCOL

    pos = np.empty((B, 2), dtype=np.int64)
    for c, (b, s) in enumerate(order):
        pos[b, s] = c
    return order, tuple(subgroups), pos


def _build(subgroups):
    nc = bacc.Bacc(
        "TRN2", target_bir_lowering=False, debug=False, num_devices=NCORES
    )
    dt = mybir.dt

    w_ins = [
        nc.dram_tensor(
            f"w_in{i}", [P, KEDGE[i + 1] - KEDGE[i], FCH, CH], dt.bfloat16,
            kind="ExternalInput",
        )
        for i in range(NPIECE)
    ]
    f_ins = [
        nc.dram_tensor(
            f"f_in{i}", [P, FCH, FEDGE[i + 1] - FEDGE[i]], dt.bfloat16,
            kind="ExternalInput",
        )
        for i in range(NPIECE)
    ]
    eb_d = nc.dram_tensor("eb", [2, K * CH], dt.bfloat16, kind="ExternalInput")
    flags_d = nc.dram_tensor("flags", [2, NCOL], dt.bfloat16, kind="ExternalInput")
    sof_d = nc.dram_tensor("sof", [P, NCHUNK], dt.int32, kind="ExternalInput")
    outT = nc.dram_tensor("outT", [CH, B], dt.float32, kind="ExternalOutput")

    with tile.TileContext(nc) as tc, ExitStack() as ctx:
        const_pool = ctx.enter_context(tc.tile_pool(name="const", bufs=1))
        psum_pool = ctx.enter_context(tc.tile_pool(name="psum", bufs=1, space="PSUM"))

        # small tensors on the SWDGE queue (independent descriptor generator)
        sof_sb = const_pool.tile([P, NCHUNK], dt.int32)
        nc.gpsimd.dma_start(out=sof_sb[:], in_=sof_d.ap())
        srow_sb = const_pool.tile([P, B], dt.int32)
        nc.gpsimd.iota(srow_sb[:], pattern=[[1, B]], base=0, channel_multiplier=0)
        eb_sb = const_pool.tile([2, K * CH], dt.bfloat16)
        nc.gpsimd.dma_start(out=eb_sb[:], in_=eb_d.ap())
        flags_sb = const_pool.tile([2, NCOL], dt.bfloat16)
        nc.gpsimd.dma_start(out=flags_sb[:], in_=flags_d.ap())
        ident_sb = const_pool.tile([CH, CH], dt.bfloat16)
        make_identity(nc, ident_sb[:])

        # one-hot pairing tiles S_c[p, b] = (sampleof(128c+p) == b), bf16
        s_sb = const_pool.tile([P, NCHUNK * B], dt.bfloat16)
        for c in range(NCHUNK):
            nc.vector.tensor_tensor(
                out=s_sb[:, c * B : (c + 1) * B],
                in0=sof_sb[:, c : c + 1].to_broadcast([P, B]),
                in1=srow_sb[:],
                op=mybir.AluOpType.is_equal,
            )

        # bulk streams: weights on sync, features on scalar
        w_sb = []
        f_sb = []
        for i in range(NPIECE):
            nk = KEDGE[i + 1] - KEDGE[i]
            wt = const_pool.tile([P, nk * FCH * CH], dt.bfloat16, name=f"wt{i}")
            nc.sync.dma_start(
                out=wt[:], in_=w_ins[i].ap().rearrange("p k f c -> p (k f c)")
            )
            w_sb.append(wt)
            nf = FEDGE[i + 1] - FEDGE[i]
            ft = const_pool.tile([P, FCH * nf], dt.bfloat16, name=f"ft{i}")
            nc.scalar.dma_start(
                out=ft[:], in_=f_ins[i].ap().rearrange("p f c -> p (f c)")
            )
            f_sb.append(ft)

        # psum: main accumulator (banded), transpose staging, pairing output
        acc = psum_pool.tile([P, BINW], dt.float32, space="PSUM")
        tpsum = psum_pool.tile([P, NCHUNK * CH], dt.bfloat16, space="PSUM")
        pout = psum_pool.tile([P, B], dt.float32, space="PSUM")

        rows_sb = const_pool.tile([CH, NCOL], dt.bfloat16)
        rcT_sb = const_pool.tile([P, NCHUNK * CH], dt.bfloat16)

        def piece_of(c0):
            for i in range(NPIECE):
                if FEDGE[i] <= c0 < FEDGE[i + 1]:
                    return i
            raise AssertionError

        by_piece_band = {}
        for k, c0, n in subgroups:
            by_piece_band.setdefault(
                (piece_of(c0), c0 // BINW), []
            ).append((k, c0, n))

        def emit_mains(pc):
            bands = sorted(b for (p_, b) in by_piece_band if p_ == pc)
            lists = [list(by_piece_band[(pc, b)]) for b in bands]
            li = 0
            while any(lists):
                if lists[li % len(lists)]:
                    k, c0, n = lists[li % len(lists)].pop(0)
                    band = c0 // BINW
                    off = c0 % BINW
                    poff = c0 - FEDGE[pc]
                    npc = FEDGE[pc + 1] - FEDGE[pc]
                    wc = next(
                        i for i in range(NPIECE) if KEDGE[i] <= k < KEDGE[i + 1]
                    )
                    kk = k - KEDGE[wc]
                    for ch in range(FCH):
                        nc.tensor.matmul(
                            out=acc[32 * band : 32 * band + CH, off : off + n],
                            lhsT=w_sb[wc][
                                :, (kk * FCH + ch) * CH : (kk * FCH + ch + 1) * CH
                            ],
                            rhs=f_sb[pc][:, ch * npc + poff : ch * npc + poff + n],
                            start=(ch == 0),
                            stop=False,
                            tile_position=(0, 32 * band),
                        )
                    # row-640 extra (all pairs) + bias (slot-A pairs), K=2
                    nc.tensor.matmul(
                        out=acc[32 * band : 32 * band + CH, off : off + n],
                        lhsT=eb_sb[0:2, k * CH : (k + 1) * CH],
                        rhs=flags_sb[0:2, c0 : c0 + n],
                        start=False,
                        stop=True,
                        tile_position=(0, 32 * band),
                    )
                li += 1

        def emit_fold(band):
            # psum band -> flat bf16 rows (partition-shifted psum read)
            nc.vector.tensor_copy(
                rows_sb[:, band * BINW : (band + 1) * BINW],
                acc[32 * band : 32 * band + CH, :],
            )

        def emit_transposes(cs):
            for c in cs:
                nc.tensor.transpose(
                    out=tpsum[:, c * CH : (c + 1) * CH],
                    in_=rows_sb[:, c * P : (c + 1) * P],
                    identity=ident_sb[:],
                )

        def emit_tcopy(g):
            nc.vector.tensor_copy(
                rcT_sb[:, g * 4 * CH : (g + 1) * 4 * CH],
                tpsum[:, g * 4 * CH : (g + 1) * 4 * CH],
            )

        def emit_pairing(cs):
            for c in cs:
                band = c % NBAND
                for h in range(2):
                    nc.tensor.matmul(
                        out=pout[32 * band : 32 * band + CH, h * BINW : (h + 1) * BINW],
                        lhsT=rcT_sb[:, c * CH : (c + 1) * CH],
                        rhs=s_sb[:, c * B + h * BINW : c * B + (h + 1) * BINW],
                        start=(c // NBAND == 0),
                        stop=(c // NBAND == NBAND - 1),
                        tile_position=(0, 32 * band),
                        skip_group_check=True,
                    )

        # pipeline: mains by piece; band work as its columns complete
        emit_mains(0)          # cols 0..682
        emit_mains(1)          # cols 683..1365
        emit_fold(0)
        emit_fold(1)
        emit_transposes(range(0, 8))
        emit_tcopy(0)
        emit_tcopy(1)
        emit_pairing(range(0, 8))
        emit_mains(2)          # cols 1366..2047
        emit_fold(2)
        emit_fold(3)
        emit_transposes(range(8, 16))
        emit_tcopy(2)
        emit_tcopy(3)
        emit_pairing(range(8, 16))

        # sum the 4 pairing bands (partition-shifted psum reads)
        t0_sb = const_pool.tile([CH, B], dt.float32)
        nc.vector.tensor_copy(t0_sb[:], pout[0:CH, :])
        t1_sb = const_pool.tile([CH, B], dt.float32)
        nc.vector.tensor_tensor(
            out=t1_sb[:], in0=pout[CH : 2 * CH, :], in1=t0_sb[:],
            op=mybir.AluOpType.add,
        )
        t2_sb = const_pool.tile([CH, B], dt.float32)
        nc.vector.tensor_tensor(
            out=t2_sb[:], in0=pout[2 * CH : 3 * CH, :], in1=t1_sb[:],
            op=mybir.AluOpType.add,
        )
        outT_sb = const_pool.tile([CH, B], dt.float32)
        nc.vector.tensor_tensor(
            out=outT_sb[:], in0=pout[3 * CH : 4 * CH, :], in1=t2_sb[:],
            op=mybir.AluOpType.add,
        )
        nc.sync.dma_start(out=outT.ap(), in_=outT_sb[:])

    nc.compile()
    return nc


def kernel(piece_positions, king_positions, input_weights, bias):
    global LAST_RESULTS, LAST_EXEC_NS

    p_flat = np.asarray(piece_positions).reshape(B, F)
    w_full = np.ascontiguousarray(np.asarray(input_weights), dtype=np.float32)
    bias_np = np.asarray(bias, dtype=np.float32)

    order, subgroups, pos = _prep(king_positions)

    if subgroups not in _cache:
        _cache[subgroups] = _build(subgroups)
    nc = _cache[subgroups]

    w_bf = w_full.astype(BF16)  # (K, 641, C)

    # features in pair-column order: (2048, 640) -> (128, 5, 2048) bf16
    bs = np.array([b for b, _ in order], dtype=np.int64)
    featsT = (
        p_flat[bs].astype(np.float32).reshape(NCOL, FCH, 128).transpose(2, 1, 0)
    ).astype(BF16)
    f_pieces = [
        np.ascontiguousarray(featsT[:, :, FEDGE[i] : FEDGE[i + 1]])
        for i in range(NPIECE)
    ]

    # flags: row0 = 1 (row-640 extra), row1 = slot-A indicator (bias once)
    flags = np.zeros((2, NCOL), dtype=np.float32)
    flags[0, :] = 1.0
    flags[1, :] = np.array([1.0 if s == 0 else 0.0 for _, s in order])
    flags = flags.astype(BF16)

    # pairing metadata: sample index of each pair column, and iota row
    sof = np.empty((P, NCHUNK), dtype=np.int32)
    for c0, (b, s) in enumerate(order):
        sof[c0 % P, c0 // P] = b

    in_maps = []
    for c in range(NCORES):
        chs = slice(c * CH, (c + 1) * CH)
        w_c = (
            w_bf[:, :F, chs].reshape(K, FCH, 128, CH).transpose(2, 0, 1, 3)
        )  # (128, K, FCH, CH)
        eb_c = np.zeros((2, K, CH), dtype=np.float32)
        eb_c[0] = w_full[:, F, chs]
        eb_c[1] = bias_np[chs][None, :]
        m = {
            "eb": np.ascontiguousarray(eb_c.reshape(2, K * CH)).astype(BF16),
            "flags": flags,
            "sof": sof,
        }
        for i in range(NPIECE):
            m[f"w_in{i}"] = np.ascontiguousarray(w_c[:, KEDGE[i] : KEDGE[i + 1]])
            m[f"f_in{i}"] = f_pieces[i]
        in_maps.append(m)

    do_trace = bool(int(os.environ.get("KERNEL_TRACE", "0")))
    trace_kw = dict(
        trace=do_trace, trace_cores=list(range(NCORES)) if do_trace else None
    )

    res = run_bass_kernel_spmd(nc, in_maps, list(range(NCORES)), **trace_kw)

    LAST_RESULTS = [res]
    LAST_EXEC_NS = res.exec_time_ns

    out = np.empty((B, C), dtype=np.float32)
    for c in range(NCORES):
        out[:, c * CH : (c + 1) * CH] = res.results[c]["outT"].T
    return out
